# revision 5
# baseline (speedup 1.0000x reference)
"""Antialiased bicubic 4x downscale (blur -> bicubic/2, twice) on 8 TRN2 cores.

The whole chain is linear and separable: every stage is M_H (x) M_W acting on
the H/W axes, so the composition collapses to a single 1024->256 banded matrix
T applied on both sides: out = T @ X @ T^T per (batch, channel) image.

Sharding: pure data parallel - batch 16 -> 2 per core, 6 images/core.

The kernel is HBM-bound: 24 MiB of x per core streams at line rate
(~365-420 GB/s measured) on the gpsimd SWDGE ring; descriptors are kept
>= 2 KB (smaller runs measurably collapse the end-of-stream drain rate).
The first chunk is small (512 KB) so its bytes land while the next chunk's
descriptors are still being emitted.  All compute (pass 1 f32r matmuls
exploiting T's band sparsity, PE transposes, bf16 pass 2) hides under the
stream; the work remaining after the last byte is minimized:

  The last image arrives as ch0 (cols 0-511, processed classically early),
  then cols 512-1023 in four row-block chunks (pc 0-3, 4-5, 6, 7).  Those
  columns take a transposed pass 1 entirely in f32r (no casts): Yt[qc 4-7]
  accumulate with the arriving x row-blocks as stationary against banded
  Tt windows, into three PSUM region tiles split at the window boundaries
  (cols [0,126), [126,190), [190,256)) so each region is evacuated to SBUF
  as soon as its last contributing row-block (pc3 / pc5 / pc7) is in.
  z is split at column 126: z[:, :, 0:126] (qc 0-3 only) is computed and
  stored early; z[:, 0, 126:256] needs Yt cols 0:128 (regions A+B, done
  after pc5) and is computed + stored while pc6/pc7 still stream.  After
  the last byte only remain: 4 windowed f32r matmuls (pc7), one small
  PSUM evac, 4 pass-2 matmuls (z[:, 1, 126:256]), one evac and one
  128x520B store.
"""

import numpy as np
import ml_dtypes

import concourse.bacc as bacc
import concourse.mybir as mybir
import concourse.tile as tile
from concourse.bass_utils import run_bass_kernel_spmd

SIGMA = 0.66
BICUBIC_W = np.array([-0.09375, 0.59375, 0.59375, -0.09375], dtype=np.float64)

N_CORES = 8
B, C, H, W = 16, 3, 1024, 1024
HO = H // 4
IMGS = (B // N_CORES) * C  # 6 images per core

F32 = mybir.dt.float32
F32R = mybir.dt.float32r
BF16 = mybir.dt.bfloat16

# z column split for the last image: z[:, 0:ZCUT] depends only on qc 0-3
# (x cols 0-511) and is stored early; z[:, ZCUT:] additionally gets the
# qc 4-7 contributions added after the stream ends.
ZCUT = 126


def _gauss_matrix(n):
    x = np.arange(3, dtype=np.float32) - np.float32(1.0)
    k = np.exp(np.float32(-0.5) * (x / np.float32(SIGMA)) ** 2)
    k = (k / k.sum()).astype(np.float64)
    G = np.zeros((n, n))
    for t in range(3):
        G += k[t] * np.eye(n, n, t - 1)
    return G


def _down_matrix(n):
    # out[i] = sum_t w[t] * x[clamp(2i + t - 1, 0, n-1)]
    m = n // 2
    D = np.zeros((m, n))
    for i in range(m):
        for t in range(4):
            j = min(max(2 * i + t - 1, 0), n - 1)
            D[i, j] += BICUBIC_W[t]
    return D


def build_T():
    T = _down_matrix(H // 2) @ _gauss_matrix(H // 2) @ _down_matrix(H) @ _gauss_matrix(H)
    return T.astype(np.float32)  # [256, 1024]


def _pass1_pieces(Tt):
    """(pc, ih) pairs where Tt[128pc:128pc+128, 128ih:128ih+128] is nonzero."""
    pieces = []
    for ih in range(2):
        for pc in range(8):
            if np.any(Tt[128 * pc : 128 * (pc + 1), 128 * ih : 128 * (ih + 1)]):
                pieces.append((pc, ih))
    return pieces


def _pc_windows(Tt):
    """Per row-block pc, the [a, b) span of nonzero columns of Tt."""
    wins = []
    for pc in range(8):
        nz = np.nonzero(np.any(Tt[128 * pc : 128 * (pc + 1), :] != 0, axis=0))[0]
        wins.append((int(nz.min()), int(nz.max()) + 1))
    return wins


def _build_graph():
    Tt = build_T().T  # [1024, 256]
    pieces = _pass1_pieces(Tt)
    pcs_by_ih = [[pc for (pc, ih2) in pieces if ih2 == ih] for ih in range(2)]
    wins = _pc_windows(Tt)

    nc = bacc.Bacc("TRN2", target_bir_lowering=False, debug=False)
    x = nc.dram_tensor("x", [IMGS, H, W], F32R, kind="ExternalInput").ap()
    # tb is host-prearranged to the SBUF layout: tb[p, c, n] = Tt[128c+p, n]
    tb = nc.dram_tensor("tb", [128, 8, HO], BF16, kind="ExternalInput").ap()
    eye = nc.dram_tensor("eye", [128, 128], BF16, kind="ExternalInput").ap()
    # out in SBUF layout [p, img, c, j] = Z[img, 128c+p, j]; host unscrambles
    out = nc.dram_tensor("out", [128, IMGS, 2, HO], F32, kind="ExternalOutput").ap()

    with tile.TileContext(nc) as tc:
        with (
            tc.tile_pool(name="const", bufs=1) as cpool,
            tc.tile_pool(name="xin", bufs=4) as xpool,
            tc.tile_pool(name="ysb", bufs=2) as ypool,
            tc.tile_pool(name="ytsb", bufs=2) as ytpool,
            tc.tile_pool(name="zout", bufs=2) as zpool,
            tc.tile_pool(name="psy", bufs=4, space="PSUM") as psy,
            tc.tile_pool(name="pst", bufs=2, space="PSUM") as pst,
            tc.tile_pool(name="ps2", bufs=2, space="PSUM") as ps2,
        ):
            ttb = cpool.tile([128, 8, HO], BF16, tag="ttb")
            nc.scalar.dma_start(out=ttb[:], in_=tb)
            ident = cpool.tile([128, 128], BF16, tag="ident")
            nc.scalar.dma_start(out=ident[:], in_=eye)
            # f32r copy of Tt for pass 1 stationary, cast on-chip
            tt = cpool.tile([128, 8, HO], F32R, tag="tt")
            nc.vector.tensor_copy(tt[:], ttb[:])

            def p1mm(yq, pc, ih, xap, start, stop):
                nc.tensor.matmul(
                    yq,
                    tt[:, pc, 128 * ih : 128 * (ih + 1)],
                    xap,
                    start=start,
                    stop=stop,
                )

            for img in range(IMGS):
                xt = xpool.tile([128, 8, W], F32R, tag="xt", name=f"xt{img}")
                xr = x[img].rearrange("(c p) w -> p c w", p=128)

                y_sb = ypool.tile([128, 2, W], BF16)
                yt_sb = ytpool.tile([128, 8, HO], BF16)
                z = zpool.tile([128, 2, HO], F32, tag="zout", name=f"z{img}")

                def evac(dst, src, ih):
                    if ih == 0:
                        nc.vector.tensor_copy(dst, src)
                    else:
                        nc.scalar.copy(dst, src)

                def transposes(ih, qc0, nqc, tag):
                    tp = pst.tile(
                        [128, 512], BF16, tag="pst",
                        name=f"tp{img}_{tag}_{ih}",
                    )
                    for s in range(nqc):
                        qc = qc0 + s
                        nc.tensor.matmul(
                            tp[:, 128 * s : 128 * (s + 1)],
                            y_sb[:, ih, 128 * qc : 128 * (qc + 1)],
                            ident[:],
                            is_transpose=True,
                            start=(s == 0),
                            stop=(s == nqc - 1),
                        )
                    dst = yt_sb[:, qc0 : qc0 + nqc, 128 * ih : 128 * (ih + 1)]
                    tsrc = tp[:, 0 : 128 * nqc].rearrange("p (s w) -> p s w", s=nqc)
                    evac(dst, tsrc, ih)

                def p2mm(acc, qc, ih, jslice, start, stop):
                    nc.tensor.matmul(
                        acc,
                        yt_sb[:, qc, 128 * ih : 128 * (ih + 1)],
                        ttb[:, qc, jslice],
                        start=start,
                        stop=stop,
                    )

                if img < IMGS - 1:
                    # row-block chunked loads; 4 KB descriptors.  The very
                    # first chunk is small so its bytes hit SBUF while the
                    # next chunk's descriptors are still being emitted.
                    if img == 0:
                        nc.gpsimd.dma_start(out=xt[:, 0:1], in_=xr[:, 0:1])
                        nc.gpsimd.dma_start(out=xt[:, 1:4], in_=xr[:, 1:4])
                    else:
                        nc.gpsimd.dma_start(out=xt[:, 0:4], in_=xr[:, 0:4])
                    nc.gpsimd.dma_start(out=xt[:, 4:8], in_=xr[:, 4:8])
                    for ch in range(2):
                        for ih in range(2):
                            yq = psy.tile(
                                [128, 512], F32, tag="psy",
                                name=f"psy{img}_{ch}_{ih}",
                            )
                            pcs = pcs_by_ih[ih]
                            for k, pc in enumerate(pcs):
                                p1mm(yq[:], pc, ih,
                                     xt[:, pc, 512 * ch : 512 * (ch + 1)],
                                     k == 0, k == len(pcs) - 1)
                            evac(y_sb[:, ih, 512 * ch : 512 * (ch + 1)], yq[:], ih)
                        for ih in range(2):
                            transposes(ih, 4 * ch, 4, f"c{ch}")
                    for ih in range(2):
                        acc = ps2.tile([128, HO], F32, tag="ps2",
                                       name=f"ps2_{img}_{ih}")
                        for qc in range(8):
                            p2mm(acc[:], qc, ih, slice(0, HO), qc == 0, qc == 7)
                        evac(z[:, ih, :], acc[:], ih)
                    nc.sync.dma_start(out=out[:, img], in_=z[:])
                else:
                    # last image: ch0 (cols 0-511), then cols 512-1023 in
                    # five row-block chunks (2 KB source descriptors; the
                    # late chunks are single row-blocks so their dependent
                    # matmuls retire chunk-by-chunk as the stream drains)
                    nc.gpsimd.dma_start(out=xt[:, :, 0:512], in_=xr[:, :, 0:512])
                    nc.gpsimd.dma_start(out=xt[:, 0:4, 512:1024],
                                        in_=xr[:, 0:4, 512:1024])
                    for pc in range(4, 8):
                        nc.gpsimd.dma_start(out=xt[:, pc : pc + 1, 512:1024],
                                            in_=xr[:, pc : pc + 1, 512:1024])

                    # ch0: classic pass 1 + transposes -> yt qc 0-3
                    for ih in range(2):
                        yq = psy.tile([128, 512], F32, tag="psy",
                                      name=f"psyL_{ih}")
                        pcs = pcs_by_ih[ih]
                        for k, pc in enumerate(pcs):
                            p1mm(yq[:], pc, ih, xt[:, pc, 0:512],
                                 k == 0, k == len(pcs) - 1)
                        evac(y_sb[:, ih, 0:512], yq[:], ih)
                    for ih in range(2):
                        transposes(ih, 0, 4, "L")

                    # zA: narrow qc 0-3 block for cols 0-125, stored early
                    for ih in range(2):
                        acc = ps2.tile([128, ZCUT], F32, tag="ps2",
                                       name=f"ps2A_{ih}")
                        for qc in range(4):
                            p2mm(acc[:], qc, ih, slice(0, ZCUT),
                                 qc == 0, qc == 3)
                        evac(z[:, ih, 0:ZCUT], acc[:], ih)
                    nc.sync.dma_start(out=out[:, img, :, 0:ZCUT],
                                      in_=z[:, :, 0:ZCUT])

                    # cols 512-1023 via transposed pass 1, entirely f32r (no
                    # casts): Yt[qc 4-7] accumulated with arriving x
                    # row-blocks as stationary against banded Tt windows,
                    # into three PSUM region tiles split at the pc4/pc6
                    # window starts so each region's accumulation closes as
                    # soon as its last contributing row-block is in
                    # (A <- pc3, B <- pc5, C <- pc7).
                    SA, SB = wins[4][0], wins[6][0]
                    ytqA = psy.tile([128, 4, SA], F32, tag="psy", name="ytqA")
                    ytqB = psy.tile([128, 4, SB - SA], F32, tag="psy",
                                    name="ytqB")
                    ytqC = psy.tile([128, 4, HO - SB], F32, tag="psy",
                                    name="ytqC")
                    regions = [(0, SA, ytqA), (SA, SB, ytqB), (SB, HO, ytqC)]

                    # start/stop flags: first/last write per region tile
                    # (emission order = pc-major, qc inner, regions inner)
                    sched = []
                    for pc in range(8):
                        a, b = wins[pc]
                        for qc in (4, 5, 6, 7):
                            for ti, (ra, rb, _t) in enumerate(regions):
                                sa, sb = max(a, ra), min(b, rb)
                                if sa < sb:
                                    sched.append((pc, qc, ti, sa, sb))
                    first_w = {}
                    last_w = {}
                    for w in sched:
                        first_w.setdefault(w[2], w)
                        last_w[w[2]] = w

                    def p1t(pcg):
                        for pc in pcg:
                            a, b = wins[pc]
                            for qc in (4, 5, 6, 7):
                                for ti, (ra, rb, t) in enumerate(regions):
                                    sa, sb = max(a, ra), min(b, rb)
                                    if sa >= sb:
                                        continue
                                    w = (pc, qc, ti, sa, sb)
                                    nc.tensor.matmul(
                                        t[:, qc - 4, sa - ra : sb - ra],
                                        xt[:, pc,
                                           512 + 128 * (qc - 4) :
                                           512 + 128 * (qc - 3)],
                                        tt[:, pc, sa:sb],
                                        start=(w == first_w[ti]),
                                        stop=(w == last_w[ti]),
                                    )

                    # z cols 126-255 accumulate per ih in separate banks;
                    # the qc3 contribution (from ch0) starts each bank early
                    zb0 = ps2.tile([128, HO - ZCUT], F32, tag="ps2",
                                   name="zb0")
                    zb1 = ps2.tile([128, HO - ZCUT], F32, tag="ps2",
                                   name="zb1")
                    nc.tensor.matmul(zb0[:], yt_sb[:, 3, 0:128],
                                     ttb[:, 3, ZCUT:HO],
                                     start=True, stop=False)
                    nc.tensor.matmul(zb1[:], yt_sb[:, 3, 128:256],
                                     ttb[:, 3, ZCUT:HO],
                                     start=True, stop=False)

                    p1t((0, 1, 2, 3))
                    nc.vector.tensor_copy(yt_sb[:, 4:8, 0:SA], ytqA[:])
                    p1t((4,))
                    p1t((5,))
                    nc.scalar.copy(yt_sb[:, 4:8, SA:SB], ytqB[:])

                    # z[:, 0, 126:] needs Yt cols 0:128 (A+B): compute and
                    # store while pc6/pc7 still stream
                    for qc in range(4, 8):
                        nc.tensor.matmul(zb0[:], yt_sb[:, qc, 0:128],
                                         ttb[:, qc, ZCUT:HO],
                                         start=False, stop=(qc == 7))
                    nc.vector.tensor_copy(z[:, 0, ZCUT:HO], zb0[:])
                    nc.sync.dma_start(out=out[:, img, 0, ZCUT:HO],
                                      in_=z[:, 0, ZCUT:HO])

                    p1t((6,))
                    p1t((7,))
                    nc.scalar.copy(yt_sb[:, 4:8, SB:HO], ytqC[:])
                    for qc in range(4, 8):
                        nc.tensor.matmul(zb1[:], yt_sb[:, qc, 128:256],
                                         ttb[:, qc, ZCUT:HO],
                                         start=False, stop=(qc == 7))
                    nc.vector.tensor_copy(z[:, 1, ZCUT:HO], zb1[:])
                    nc.sync.dma_start(out=out[:, img, 1, ZCUT:HO],
                                      in_=z[:, 1, ZCUT:HO])
    nc.compile()
    return nc


_GRAPH = None


def _get_graph():
    global _GRAPH
    if _GRAPH is None:
        _GRAPH = _build_graph()
    return _GRAPH


def run(x, **spmd_kwargs):
    x = np.ascontiguousarray(np.asarray(x, dtype=np.float32))
    assert x.shape == (B, C, H, W)
    nc = _get_graph()
    Tt = build_T().T  # [1024, 256] f32
    tb_host = np.ascontiguousarray(
        Tt.reshape(8, 128, HO).transpose(1, 0, 2)
    ).astype(ml_dtypes.bfloat16)
    eye_host = np.eye(128, dtype=ml_dtypes.bfloat16)
    per_core = B // N_CORES
    in_maps = [
        {
            "x": x[i * per_core : (i + 1) * per_core].reshape(IMGS, H, W),
            "tb": tb_host,
            "eye": eye_host,
        }
        for i in range(N_CORES)
    ]
    res = run_bass_kernel_spmd(nc, in_maps, core_ids=list(range(N_CORES)), **spmd_kwargs)
    outs = []
    for r in res.results:
        o = r["out"].transpose(1, 2, 0, 3).reshape(IMGS, 2 * 128, HO)
        outs.append(o.reshape(per_core, C, HO, HO))
    return np.concatenate(outs, axis=0), res


def kernel(x):
    out, _ = run(x)
    return out



# revision 8
# speedup vs baseline: 1.0705x; 1.0705x over previous
"""Antialiased bicubic 4x downscale (blur -> bicubic/2, twice) on 8 TRN2 cores.

The whole chain is linear and separable: every stage is M_H (x) M_W acting on
the H/W axes, so the composition collapses to a single 1024->256 banded matrix
T applied on both sides: out = T @ X @ T^T per (batch, channel) image.

Sharding: pure data parallel - batch 16 -> 2 per core, 6 images/core.

The kernel is HBM-bound: 24 MiB of x per core streams at line rate
(~365-420 GB/s measured) on the gpsimd SWDGE ring; descriptors are kept
>= 2 KB (smaller runs measurably collapse the end-of-stream drain rate).
The first chunk is small (512 KB) so its bytes land while the next chunk's
descriptors are still being emitted.  All compute (pass 1 f32r matmuls
exploiting T's band sparsity, PE transposes, bf16 pass 2) hides under the
stream; the work remaining after the last byte is minimized:

  The last image arrives as ch0 (cols 0-511, processed classically early),
  then cols 512-1023 in four row-block chunks (pc 0-3, 4-5, 6, 7).  Those
  columns take a transposed pass 1 entirely in f32r (no casts): Yt[qc 4-7]
  accumulate with the arriving x row-blocks as stationary against banded
  Tt windows, into three PSUM region tiles split at the window boundaries
  (cols [0,126), [126,190), [190,256)) so each region is evacuated to SBUF
  as soon as its last contributing row-block (pc3 / pc5 / pc7) is in.
  z is split at column 126: z[:, :, 0:126] (qc 0-3 only) is computed and
  stored early; z[:, 0, 126:256] needs Yt cols 0:128 (regions A+B, done
  after pc5) and is computed + stored while pc6/pc7 still stream.  After
  the last byte only remain: 4 windowed f32r matmuls (pc7), one small
  PSUM evac, 4 pass-2 matmuls (z[:, 1, 126:256]), one evac and one
  128x520B store.
"""

import numpy as np
import ml_dtypes

import concourse.bacc as bacc
import concourse.mybir as mybir
import concourse.tile as tile
from concourse.bass_utils import run_bass_kernel_spmd

SIGMA = 0.66
BICUBIC_W = np.array([-0.09375, 0.59375, 0.59375, -0.09375], dtype=np.float64)

N_CORES = 8
B, C, H, W = 16, 3, 1024, 1024
HO = H // 4
IMGS = (B // N_CORES) * C  # 6 images per core

F32 = mybir.dt.float32
F32R = mybir.dt.float32r
BF16 = mybir.dt.bfloat16

# z column split for the last image: z[:, 0:ZCUT] depends only on qc 0-3
# (x cols 0-511) and is stored early; z[:, ZCUT:] additionally gets the
# qc 4-7 contributions added after the stream ends.
ZCUT = 126


def _gauss_matrix(n):
    x = np.arange(3, dtype=np.float32) - np.float32(1.0)
    k = np.exp(np.float32(-0.5) * (x / np.float32(SIGMA)) ** 2)
    k = (k / k.sum()).astype(np.float64)
    G = np.zeros((n, n))
    for t in range(3):
        G += k[t] * np.eye(n, n, t - 1)
    return G


def _down_matrix(n):
    # out[i] = sum_t w[t] * x[clamp(2i + t - 1, 0, n-1)]
    m = n // 2
    D = np.zeros((m, n))
    for i in range(m):
        for t in range(4):
            j = min(max(2 * i + t - 1, 0), n - 1)
            D[i, j] += BICUBIC_W[t]
    return D


def build_T():
    T = _down_matrix(H // 2) @ _gauss_matrix(H // 2) @ _down_matrix(H) @ _gauss_matrix(H)
    return T.astype(np.float32)  # [256, 1024]


def _pass1_pieces(Tt):
    """(pc, ih) pairs where Tt[128pc:128pc+128, 128ih:128ih+128] is nonzero."""
    pieces = []
    for ih in range(2):
        for pc in range(8):
            if np.any(Tt[128 * pc : 128 * (pc + 1), 128 * ih : 128 * (ih + 1)]):
                pieces.append((pc, ih))
    return pieces


def _pc_windows(Tt):
    """Per row-block pc, the [a, b) span of nonzero columns of Tt."""
    wins = []
    for pc in range(8):
        nz = np.nonzero(np.any(Tt[128 * pc : 128 * (pc + 1), :] != 0, axis=0))[0]
        wins.append((int(nz.min()), int(nz.max()) + 1))
    return wins


def _build_graph():
    Tt = build_T().T  # [1024, 256]
    pieces = _pass1_pieces(Tt)
    pcs_by_ih = [[pc for (pc, ih2) in pieces if ih2 == ih] for ih in range(2)]
    wins = _pc_windows(Tt)

    nc = bacc.Bacc("TRN2", target_bir_lowering=False, debug=False)
    x = nc.dram_tensor("x", [IMGS, H, W], F32R, kind="ExternalInput").ap()
    # tb is host-prearranged to the SBUF layout: tb[p, c, n] = Tt[128c+p, n]
    tb = nc.dram_tensor("tb", [128, 8, HO], BF16, kind="ExternalInput").ap()
    eye = nc.dram_tensor("eye", [128, 128], BF16, kind="ExternalInput").ap()
    # out in SBUF layout [p, img, c, j] = Z[img, 128c+p, j]; host unscrambles
    out = nc.dram_tensor("out", [128, IMGS, 2, HO], F32, kind="ExternalOutput").ap()

    with tile.TileContext(nc) as tc:
        with (
            tc.tile_pool(name="const", bufs=1) as cpool,
            tc.tile_pool(name="xin", bufs=4) as xpool,
            tc.tile_pool(name="ysb", bufs=2) as ypool,
            tc.tile_pool(name="ytsb", bufs=2) as ytpool,
            tc.tile_pool(name="zout", bufs=2) as zpool,
            tc.tile_pool(name="psy", bufs=4, space="PSUM") as psy,
            tc.tile_pool(name="pst", bufs=2, space="PSUM") as pst,
            tc.tile_pool(name="ps2", bufs=2, space="PSUM") as ps2,
        ):
            ttb = cpool.tile([128, 8, HO], BF16, tag="ttb")
            nc.scalar.dma_start(out=ttb[:], in_=tb)
            ident = cpool.tile([128, 128], BF16, tag="ident")
            nc.scalar.dma_start(out=ident[:], in_=eye)
            # f32r copy of Tt for pass 1 stationary, cast on-chip
            tt = cpool.tile([128, 8, HO], F32R, tag="tt")
            nc.vector.tensor_copy(tt[:], ttb[:])

            def p1mm(yq, pc, ih, xap, start, stop):
                nc.tensor.matmul(
                    yq,
                    tt[:, pc, 128 * ih : 128 * (ih + 1)],
                    xap,
                    start=start,
                    stop=stop,
                )

            for img in range(IMGS):
                xt = xpool.tile([128, 8, W], F32R, tag="xt", name=f"xt{img}")
                xr = x[img].rearrange("(c p) w -> p c w", p=128)

                y_sb = ypool.tile([128, 2, W], BF16)
                yt_sb = ytpool.tile([128, 8, HO], BF16)
                z = zpool.tile([128, 2, HO], F32, tag="zout", name=f"z{img}")

                def evac(dst, src, ih):
                    if ih == 0:
                        nc.vector.tensor_copy(dst, src)
                    else:
                        nc.scalar.copy(dst, src)

                def transposes(ih, qc0, nqc, tag):
                    tp = pst.tile(
                        [128, 512], BF16, tag="pst",
                        name=f"tp{img}_{tag}_{ih}",
                    )
                    for s in range(nqc):
                        qc = qc0 + s
                        nc.tensor.matmul(
                            tp[:, 128 * s : 128 * (s + 1)],
                            y_sb[:, ih, 128 * qc : 128 * (qc + 1)],
                            ident[:],
                            is_transpose=True,
                            start=(s == 0),
                            stop=(s == nqc - 1),
                        )
                    dst = yt_sb[:, qc0 : qc0 + nqc, 128 * ih : 128 * (ih + 1)]
                    tsrc = tp[:, 0 : 128 * nqc].rearrange("p (s w) -> p s w", s=nqc)
                    evac(dst, tsrc, ih)

                def p2mm(acc, qc, ih, jslice, start, stop):
                    nc.tensor.matmul(
                        acc,
                        yt_sb[:, qc, 128 * ih : 128 * (ih + 1)],
                        ttb[:, qc, jslice],
                        start=start,
                        stop=stop,
                    )

                if img < IMGS - 1:
                    # row-block chunked loads; 4 KB descriptors.  The very
                    # first chunk is small so its bytes hit SBUF while the
                    # next chunk's descriptors are still being emitted.
                    if img == 0:
                        nc.gpsimd.dma_start(out=xt[:, 0:1], in_=xr[:, 0:1])
                        nc.gpsimd.dma_start(out=xt[:, 1:4], in_=xr[:, 1:4])
                    else:
                        nc.gpsimd.dma_start(out=xt[:, 0:4], in_=xr[:, 0:4])
                    nc.gpsimd.dma_start(out=xt[:, 4:8], in_=xr[:, 4:8])
                    for ch in range(2):
                        for ih in range(2):
                            yq = psy.tile(
                                [128, 512], F32, tag="psy",
                                name=f"psy{img}_{ch}_{ih}",
                            )
                            pcs = pcs_by_ih[ih]
                            for k, pc in enumerate(pcs):
                                p1mm(yq[:], pc, ih,
                                     xt[:, pc, 512 * ch : 512 * (ch + 1)],
                                     k == 0, k == len(pcs) - 1)
                            evac(y_sb[:, ih, 512 * ch : 512 * (ch + 1)], yq[:], ih)
                        for ih in range(2):
                            transposes(ih, 4 * ch, 4, f"c{ch}")
                    for ih in range(2):
                        acc = ps2.tile([128, HO], F32, tag="ps2",
                                       name=f"ps2_{img}_{ih}")
                        for qc in range(8):
                            p2mm(acc[:], qc, ih, slice(0, HO), qc == 0, qc == 7)
                        evac(z[:, ih, :], acc[:], ih)
                    nc.sync.dma_start(out=out[:, img], in_=z[:])
                else:
                    # last image: ch0 (cols 0-511), then cols 512-1023 in
                    # three row-block chunks (2 KB source descriptors;
                    # single-row-block chunks measure ~10x descriptor
                    # imbalance across SDMA engines, so >= 2 row blocks)
                    nc.gpsimd.dma_start(out=xt[:, :, 0:512], in_=xr[:, :, 0:512])
                    nc.gpsimd.dma_start(out=xt[:, 0:4, 512:1024],
                                        in_=xr[:, 0:4, 512:1024])
                    nc.gpsimd.dma_start(out=xt[:, 4:6, 512:1024],
                                        in_=xr[:, 4:6, 512:1024])
                    nc.gpsimd.dma_start(out=xt[:, 6:8, 512:1024],
                                        in_=xr[:, 6:8, 512:1024])

                    # ch0: classic pass 1 + transposes -> yt qc 0-3
                    for ih in range(2):
                        yq = psy.tile([128, 512], F32, tag="psy",
                                      name=f"psyL_{ih}")
                        pcs = pcs_by_ih[ih]
                        for k, pc in enumerate(pcs):
                            p1mm(yq[:], pc, ih, xt[:, pc, 0:512],
                                 k == 0, k == len(pcs) - 1)
                        evac(y_sb[:, ih, 0:512], yq[:], ih)
                    for ih in range(2):
                        transposes(ih, 0, 4, "L")

                    # zA: narrow qc 0-3 block for cols 0-125, stored early
                    for ih in range(2):
                        acc = ps2.tile([128, ZCUT], F32, tag="ps2",
                                       name=f"ps2A_{ih}")
                        for qc in range(4):
                            p2mm(acc[:], qc, ih, slice(0, ZCUT),
                                 qc == 0, qc == 3)
                        evac(z[:, ih, 0:ZCUT], acc[:], ih)
                    nc.sync.dma_start(out=out[:, img, :, 0:ZCUT],
                                      in_=z[:, :, 0:ZCUT])

                    # cols 512-1023 via transposed pass 1, entirely f32r (no
                    # casts): Yt[qc 4-7] accumulated with arriving x
                    # row-blocks as stationary against banded Tt windows,
                    # into three PSUM region tiles split at the pc4/pc6
                    # window starts so each region's accumulation closes as
                    # soon as its last contributing row-block is in
                    # (A <- pc3, B <- pc5, C <- pc7).
                    SA, SB = wins[4][0], wins[6][0]
                    ytqA = psy.tile([128, 4, SA], F32, tag="psy", name="ytqA")
                    ytqB = psy.tile([128, 4, SB - SA], F32, tag="psy",
                                    name="ytqB")
                    ytqC = psy.tile([128, 4, HO - SB], F32, tag="psy",
                                    name="ytqC")
                    regions = [(0, SA, ytqA), (SA, SB, ytqB), (SB, HO, ytqC)]

                    # start/stop flags: first/last write per region tile
                    # (emission order = pc-major, qc inner, regions inner)
                    sched = []
                    for pc in range(8):
                        a, b = wins[pc]
                        for qc in (4, 5, 6, 7):
                            for ti, (ra, rb, _t) in enumerate(regions):
                                sa, sb = max(a, ra), min(b, rb)
                                if sa < sb:
                                    sched.append((pc, qc, ti, sa, sb))
                    first_w = {}
                    last_w = {}
                    for w in sched:
                        first_w.setdefault(w[2], w)
                        last_w[w[2]] = w

                    def p1t(pcg):
                        for pc in pcg:
                            a, b = wins[pc]
                            for qc in (4, 5, 6, 7):
                                for ti, (ra, rb, t) in enumerate(regions):
                                    sa, sb = max(a, ra), min(b, rb)
                                    if sa >= sb:
                                        continue
                                    w = (pc, qc, ti, sa, sb)
                                    nc.tensor.matmul(
                                        t[:, qc - 4, sa - ra : sb - ra],
                                        xt[:, pc,
                                           512 + 128 * (qc - 4) :
                                           512 + 128 * (qc - 3)],
                                        tt[:, pc, sa:sb],
                                        start=(w == first_w[ti]),
                                        stop=(w == last_w[ti]),
                                    )

                    # z cols 126-255 accumulate per ih in separate banks;
                    # the qc3 contribution (from ch0) starts each bank early
                    zb0 = ps2.tile([128, HO - ZCUT], F32, tag="ps2",
                                   name="zb0")
                    zb1 = ps2.tile([128, HO - ZCUT], F32, tag="ps2",
                                   name="zb1")
                    nc.tensor.matmul(zb0[:], yt_sb[:, 3, 0:128],
                                     ttb[:, 3, ZCUT:HO],
                                     start=True, stop=False)
                    nc.tensor.matmul(zb1[:], yt_sb[:, 3, 128:256],
                                     ttb[:, 3, ZCUT:HO],
                                     start=True, stop=False)

                    p1t((0, 1, 2, 3))
                    nc.vector.tensor_copy(yt_sb[:, 4:8, 0:SA], ytqA[:])
                    p1t((4, 5))
                    nc.scalar.copy(yt_sb[:, 4:8, SA:SB], ytqB[:])

                    # z[:, 0, 126:] needs Yt cols 0:128 (A+B): compute and
                    # store while pc6/pc7 still stream
                    for qc in range(4, 8):
                        nc.tensor.matmul(zb0[:], yt_sb[:, qc, 0:128],
                                         ttb[:, qc, ZCUT:HO],
                                         start=False, stop=(qc == 7))
                    nc.vector.tensor_copy(z[:, 0, ZCUT:HO], zb0[:])
                    nc.sync.dma_start(out=out[:, img, 0, ZCUT:HO],
                                      in_=z[:, 0, ZCUT:HO])

                    p1t((6, 7))
                    nc.scalar.copy(yt_sb[:, 4:8, SB:HO], ytqC[:])
                    for qc in range(4, 8):
                        nc.tensor.matmul(zb1[:], yt_sb[:, qc, 128:256],
                                         ttb[:, qc, ZCUT:HO],
                                         start=False, stop=(qc == 7))
                    nc.vector.tensor_copy(z[:, 1, ZCUT:HO], zb1[:])
                    nc.sync.dma_start(out=out[:, img, 1, ZCUT:HO],
                                      in_=z[:, 1, ZCUT:HO])
    nc.compile()
    return nc


_GRAPH = None


def _get_graph():
    global _GRAPH
    if _GRAPH is None:
        _GRAPH = _build_graph()
    return _GRAPH


def run(x, **spmd_kwargs):
    x = np.ascontiguousarray(np.asarray(x, dtype=np.float32))
    assert x.shape == (B, C, H, W)
    nc = _get_graph()
    Tt = build_T().T  # [1024, 256] f32
    tb_host = np.ascontiguousarray(
        Tt.reshape(8, 128, HO).transpose(1, 0, 2)
    ).astype(ml_dtypes.bfloat16)
    eye_host = np.eye(128, dtype=ml_dtypes.bfloat16)
    per_core = B // N_CORES
    in_maps = [
        {
            "x": x[i * per_core : (i + 1) * per_core].reshape(IMGS, H, W),
            "tb": tb_host,
            "eye": eye_host,
        }
        for i in range(N_CORES)
    ]
    res = run_bass_kernel_spmd(nc, in_maps, core_ids=list(range(N_CORES)), **spmd_kwargs)
    outs = []
    for r in res.results:
        o = r["out"].transpose(1, 2, 0, 3).reshape(IMGS, 2 * 128, HO)
        outs.append(o.reshape(per_core, C, HO, HO))
    return np.concatenate(outs, axis=0), res


def kernel(x):
    out, _ = run(x)
    return out



# revision 9
# speedup vs baseline: 1.1224x; 1.0485x over previous
"""Antialiased bicubic 4x downscale (blur -> bicubic/2, twice) on 8 TRN2 cores.

The whole chain is linear and separable: every stage is M_H (x) M_W acting on
the H/W axes, so the composition collapses to a single 1024->256 banded matrix
T applied on both sides: out = T @ X @ T^T per (batch, channel) image.

Sharding: pure data parallel - batch 16 -> 2 per core, 6 images/core.

The kernel is HBM-bound: 24 MiB of x per core streams at line rate (~360 GB/s
measured) on the gpsimd SWDGE ring; descriptors are kept >= 2 KB (smaller
runs measurably collapse the end-of-stream drain rate).  A tiny warmup DMA
absorbs the DMA-queue spin-up before the stream.  All compute (pass 1 f32r
matmuls exploiting T's band sparsity, PE transposes, bf16 pass 2) hides under
the stream; the work remaining after the last byte is minimized:

  The last image arrives as ch0 (cols 0-511, processed classically early),
  then cols 512-1023 in three row-block groups, cast to bf16 in-flight
  (SWDGE).  Those columns take a transposed pass 1: Yt[qc 4-7] accumulate
  directly with the arriving bf16 x row-blocks as stationary (fast weight
  load) against banded Tt windows - no evac+PE-transpose chain trails the
  stream.  The output is split at column 126: z[:, 0:126] (plus the full
  qc 0-3 partial for the rest) is computed and stored while cols 512+
  stream; after the last byte only 8 windowed matmuls, two casts, 8 small
  pass-2 matmuls, two fused add-evacs and a 130-column store remain.
"""

import numpy as np
import ml_dtypes

import concourse.bacc as bacc
import concourse.mybir as mybir
import concourse.tile as tile
from concourse.bass_utils import run_bass_kernel_spmd

SIGMA = 0.66
BICUBIC_W = np.array([-0.09375, 0.59375, 0.59375, -0.09375], dtype=np.float64)

N_CORES = 8
B, C, H, W = 16, 3, 1024, 1024
HO = H // 4
IMGS = (B // N_CORES) * C  # 6 images per core

F32 = mybir.dt.float32
F32R = mybir.dt.float32r
BF16 = mybir.dt.bfloat16

# z column split for the last image: z[:, 0:ZCUT] depends only on qc 0-3
# (x cols 0-511) and is stored early; z[:, ZCUT:] additionally gets the
# qc 4-7 contributions added after the stream ends.
ZCUT = 126


def _gauss_matrix(n):
    x = np.arange(3, dtype=np.float32) - np.float32(1.0)
    k = np.exp(np.float32(-0.5) * (x / np.float32(SIGMA)) ** 2)
    k = (k / k.sum()).astype(np.float64)
    G = np.zeros((n, n))
    for t in range(3):
        G += k[t] * np.eye(n, n, t - 1)
    return G


def _down_matrix(n):
    # out[i] = sum_t w[t] * x[clamp(2i + t - 1, 0, n-1)]
    m = n // 2
    D = np.zeros((m, n))
    for i in range(m):
        for t in range(4):
            j = min(max(2 * i + t - 1, 0), n - 1)
            D[i, j] += BICUBIC_W[t]
    return D


def build_T():
    T = _down_matrix(H // 2) @ _gauss_matrix(H // 2) @ _down_matrix(H) @ _gauss_matrix(H)
    return T.astype(np.float32)  # [256, 1024]


def _pass1_pieces(Tt):
    """(pc, ih) pairs where Tt[128pc:128pc+128, 128ih:128ih+128] is nonzero."""
    pieces = []
    for ih in range(2):
        for pc in range(8):
            if np.any(Tt[128 * pc : 128 * (pc + 1), 128 * ih : 128 * (ih + 1)]):
                pieces.append((pc, ih))
    return pieces


def _pc_windows(Tt):
    """Per row-block pc, the [a, b) span of nonzero columns of Tt."""
    wins = []
    for pc in range(8):
        nz = np.nonzero(np.any(Tt[128 * pc : 128 * (pc + 1), :] != 0, axis=0))[0]
        wins.append((int(nz.min()), int(nz.max()) + 1))
    return wins


def _build_graph():
    Tt = build_T().T  # [1024, 256]
    pieces = _pass1_pieces(Tt)
    pcs_by_ih = [[pc for (pc, ih2) in pieces if ih2 == ih] for ih in range(2)]
    wins = _pc_windows(Tt)

    nc = bacc.Bacc("TRN2", target_bir_lowering=False, debug=False)
    x = nc.dram_tensor("x", [IMGS, H, W], F32R, kind="ExternalInput").ap()
    # tb is host-prearranged to the SBUF layout: tb[p, c, n] = Tt[128c+p, n]
    tb = nc.dram_tensor("tb", [128, 8, HO], BF16, kind="ExternalInput").ap()
    eye = nc.dram_tensor("eye", [128, 128], BF16, kind="ExternalInput").ap()
    # out in SBUF layout [p, img, c, j] = Z[img, 128c+p, j]; host unscrambles
    out = nc.dram_tensor("out", [128, IMGS, 2, HO], F32, kind="ExternalOutput").ap()

    with tile.TileContext(nc) as tc:
        with (
            tc.tile_pool(name="const", bufs=1) as cpool,
            tc.tile_pool(name="xin", bufs=4) as xpool,
            tc.tile_pool(name="xbin", bufs=1) as xbpool,
            tc.tile_pool(name="ysb", bufs=2) as ypool,
            tc.tile_pool(name="ytsb", bufs=2) as ytpool,
            tc.tile_pool(name="zout", bufs=2) as zpool,
            tc.tile_pool(name="psy", bufs=4, space="PSUM") as psy,
            tc.tile_pool(name="pst", bufs=2, space="PSUM") as pst,
            tc.tile_pool(name="ps2", bufs=2, space="PSUM") as ps2,
        ):
            # tiny warmup load: spins up the SWDGE queue/engines so the real
            # stream's first bytes land sooner
            warm = cpool.tile([128, 8], F32R, tag="warm")
            nc.gpsimd.dma_start(out=warm[:], in_=x[0, 0:128, 0:8])

            ttb = cpool.tile([128, 8, HO], BF16, tag="ttb")
            nc.scalar.dma_start(out=ttb[:], in_=tb)
            ident = cpool.tile([128, 128], BF16, tag="ident")
            nc.scalar.dma_start(out=ident[:], in_=eye)
            # f32r copy of Tt for pass 1 stationary, cast on-chip
            tt = cpool.tile([128, 8, HO], F32R, tag="tt")
            nc.vector.tensor_copy(tt[:], ttb[:])

            def p1mm(yq, pc, ih, xap, start, stop):
                nc.tensor.matmul(
                    yq,
                    tt[:, pc, 128 * ih : 128 * (ih + 1)],
                    xap,
                    start=start,
                    stop=stop,
                )

            for img in range(IMGS):
                xt = xpool.tile([128, 8, W], F32R, tag="xt", name=f"xt{img}")
                xr = x[img].rearrange("(c p) w -> p c w", p=128)

                y_sb = ypool.tile([128, 2, W], BF16)
                yt_sb = ytpool.tile([128, 8, HO], BF16)
                z = zpool.tile([128, 2, HO], F32, tag="zout", name=f"z{img}")

                def evac(dst, src, ih):
                    if ih == 0:
                        nc.vector.tensor_copy(dst, src)
                    else:
                        nc.scalar.copy(dst, src)

                def transposes(ih, qc0, nqc, tag):
                    tp = pst.tile(
                        [128, 512], BF16, tag="pst",
                        name=f"tp{img}_{tag}_{ih}",
                    )
                    for s in range(nqc):
                        qc = qc0 + s
                        nc.tensor.matmul(
                            tp[:, 128 * s : 128 * (s + 1)],
                            y_sb[:, ih, 128 * qc : 128 * (qc + 1)],
                            ident[:],
                            is_transpose=True,
                            start=(s == 0),
                            stop=(s == nqc - 1),
                        )
                    dst = yt_sb[:, qc0 : qc0 + nqc, 128 * ih : 128 * (ih + 1)]
                    tsrc = tp[:, 0 : 128 * nqc].rearrange("p (s w) -> p s w", s=nqc)
                    evac(dst, tsrc, ih)

                def p2mm(acc, qc, ih, jslice, start, stop):
                    nc.tensor.matmul(
                        acc,
                        yt_sb[:, qc, 128 * ih : 128 * (ih + 1)],
                        ttb[:, qc, jslice],
                        start=start,
                        stop=stop,
                    )

                if img < IMGS - 1:
                    # row-block chunked loads; 4 KB descriptors
                    nc.gpsimd.dma_start(out=xt[:, 0:4], in_=xr[:, 0:4])
                    nc.gpsimd.dma_start(out=xt[:, 4:8], in_=xr[:, 4:8])
                    for ch in range(2):
                        for ih in range(2):
                            yq = psy.tile(
                                [128, 512], F32, tag="psy",
                                name=f"psy{img}_{ch}_{ih}",
                            )
                            pcs = pcs_by_ih[ih]
                            for k, pc in enumerate(pcs):
                                p1mm(yq[:], pc, ih,
                                     xt[:, pc, 512 * ch : 512 * (ch + 1)],
                                     k == 0, k == len(pcs) - 1)
                            evac(y_sb[:, ih, 512 * ch : 512 * (ch + 1)], yq[:], ih)
                        for ih in range(2):
                            transposes(ih, 4 * ch, 4, f"c{ch}")
                    for ih in range(2):
                        acc = ps2.tile([128, HO], F32, tag="ps2",
                                       name=f"ps2_{img}_{ih}")
                        for qc in range(8):
                            p2mm(acc[:], qc, ih, slice(0, HO), qc == 0, qc == 7)
                        evac(z[:, ih, :], acc[:], ih)
                    nc.sync.dma_start(out=out[:, img], in_=z[:])
                else:
                    # last image: ch0 (cols 0-511, f32r), then cols 512-1023
                    # in three row-block groups cast to bf16 in-flight
                    # (2 KB source descriptors throughout)
                    xb = xbpool.tile([128, 8, 512], BF16, tag="xb")
                    nc.gpsimd.dma_start(out=xt[:, :, 0:512], in_=xr[:, :, 0:512])
                    nc.gpsimd.dma_start(out=xt[:, 0:4, 512:1024],
                                        in_=xr[:, 0:4, 512:1024])
                    nc.gpsimd.dma_start(out=xt[:, 4:6, 512:1024],
                                        in_=xr[:, 4:6, 512:1024])
                    nc.gpsimd.dma_start(out=xt[:, 6:8, 512:1024],
                                        in_=xr[:, 6:8, 512:1024])

                    # ch0: classic pass 1 + transposes -> yt qc 0-3
                    for ih in range(2):
                        yq = psy.tile([128, 512], F32, tag="psy",
                                      name=f"psyL_{ih}")
                        pcs = pcs_by_ih[ih]
                        for k, pc in enumerate(pcs):
                            p1mm(yq[:], pc, ih, xt[:, pc, 0:512],
                                 k == 0, k == len(pcs) - 1)
                        evac(y_sb[:, ih, 0:512], yq[:], ih)
                    for ih in range(2):
                        transposes(ih, 0, 4, "L")

                    # cast the ch1 row-groups to bf16 as they arrive (the
                    # in-flight SWDGE cast drains erratically, so the x
                    # stream stays f32r); the final group splits across
                    # both copy engines
                    nc.vector.tensor_copy(xb[:, 0:4], xt[:, 0:4, 512:1024])
                    nc.scalar.copy(xb[:, 4:6], xt[:, 4:6, 512:1024])
                    nc.vector.tensor_copy(xb[:, 6:7], xt[:, 6:7, 512:1024])
                    nc.scalar.copy(xb[:, 7:8], xt[:, 7:8, 512:1024])

                    # cols 512-1023 via transposed pass 1: Yt[qc 4-7]
                    # accumulated with bf16 x row-blocks as stationary (FWL),
                    # banded mov windows; one accumulation group per PSUM
                    # bank (start on its first MM, stop on its last).  The
                    # ytq tiles rotate out of the psy pool so this burst sits
                    # ahead of the zA block in the PE queue, aligned with the
                    # chunk arrivals.
                    ytq = [
                        psy.tile([128, 2, HO], F32, tag="psy",
                                 name=f"ytq{g}")
                        for g in range(2)
                    ]

                    def p1t(pcg):
                        for pc in pcg:
                            a, b = wins[pc]
                            for qc in (4, 5, 6, 7):
                                g, s = divmod(qc - 4, 2)
                                nc.tensor.matmul(
                                    ytq[g][:, s, a:b],
                                    xb[:, pc, 128 * (qc - 4) : 128 * (qc - 3)],
                                    ttb[:, pc, a:b],
                                    start=(pc == 0 and s == 0),
                                    stop=(pc == 7 and s == 1),
                                )

                    p1t((0, 1, 2, 3))

                    # zA: narrow qc 0-3 block for cols 0-125, stored early
                    for ih in range(2):
                        acc = ps2.tile([128, ZCUT], F32, tag="ps2",
                                       name=f"ps2A_{ih}")
                        for qc in range(4):
                            p2mm(acc[:], qc, ih, slice(0, ZCUT),
                                 qc == 0, qc == 3)
                        evac(z[:, ih, 0:ZCUT], acc[:], ih)
                    nc.sync.dma_start(out=out[:, img, :, 0:ZCUT],
                                      in_=z[:, :, 0:ZCUT])

                    # zb accumulates z[:, :, ZCUT:] for qc 3-7 in one PSUM
                    # bank (two ih groups: single bank-clearing start on the
                    # first MM, stop on the very last)
                    zb = ps2.tile([128, 2, HO - ZCUT], F32, tag="ps2",
                                  name="zb")
                    for ih in range(2):
                        nc.tensor.matmul(
                            zb[:, ih, :],
                            yt_sb[:, 3, 128 * ih : 128 * (ih + 1)],
                            ttb[:, 3, ZCUT:HO],
                            start=(ih == 0),
                            stop=False,
                        )
                    p1t((4, 5))
                    p1t((6, 7))
                    nc.vector.tensor_copy(yt_sb[:, 4:6, :], ytq[0][:])
                    nc.scalar.copy(yt_sb[:, 6:8, :], ytq[1][:])
                    for ih in range(2):
                        for qc in range(4, 8):
                            nc.tensor.matmul(
                                zb[:, ih, :],
                                yt_sb[:, qc, 128 * ih : 128 * (ih + 1)],
                                ttb[:, qc, ZCUT:HO],
                                start=False,
                                stop=(ih == 1 and qc == 7),
                            )
                    nc.vector.tensor_copy(z[:, 0, ZCUT:HO], zb[:, 0, :])
                    nc.scalar.copy(z[:, 1, ZCUT:HO], zb[:, 1, :])
                    nc.sync.dma_start(out=out[:, img, :, ZCUT:HO],
                                      in_=z[:, :, ZCUT:HO])
    nc.compile()
    return nc


_GRAPH = None


def _get_graph():
    global _GRAPH
    if _GRAPH is None:
        _GRAPH = _build_graph()
    return _GRAPH


def run(x, **spmd_kwargs):
    x = np.ascontiguousarray(np.asarray(x, dtype=np.float32))
    assert x.shape == (B, C, H, W)
    nc = _get_graph()
    Tt = build_T().T  # [1024, 256] f32
    tb_host = np.ascontiguousarray(
        Tt.reshape(8, 128, HO).transpose(1, 0, 2)
    ).astype(ml_dtypes.bfloat16)
    eye_host = np.eye(128, dtype=ml_dtypes.bfloat16)
    per_core = B // N_CORES
    in_maps = [
        {
            "x": x[i * per_core : (i + 1) * per_core].reshape(IMGS, H, W),
            "tb": tb_host,
            "eye": eye_host,
        }
        for i in range(N_CORES)
    ]
    res = run_bass_kernel_spmd(nc, in_maps, core_ids=list(range(N_CORES)), **spmd_kwargs)
    outs = []
    for r in res.results:
        o = r["out"].transpose(1, 2, 0, 3).reshape(IMGS, 2 * 128, HO)
        outs.append(o.reshape(per_core, C, HO, HO))
    return np.concatenate(outs, axis=0), res


def kernel(x):
    out, _ = run(x)
    return out

